# revision 10
# baseline (speedup 1.0000x reference)
"""nn_NeighborSelection Trainium2 kernel.

Pipeline (8 NeuronCores, nodes sharded 12500/core):
  K1 (device): s_self/s_neigh matvec  — node_features @ [w_self|w_neigh]
               via PE fp32 matmuls, features on partitions.
  host:        concat per-core s shards; expand s_neigh[neighbors]
               (pure index permutation of a device-computed vector; no FLOPs).
  K2 (device): raw = gathered + s_self + b; leaky_relu via max(x, 0.01x);
               exact top-8 per row via DVE max8/max_index (jax tie order);
               top_scores = exp(selected) on ACT; ids extracted with a
               one-hot reduce over the 32 neighbor slots.

Selection runs on pre-exp values (exact fp32) — exp is monotone so the
top-k set/order matches the reference, and scores are exp'd after.
"""

import numpy as np

import concourse.bass as bass
import concourse.bacc as bacc
import concourse.mybir as mybir
from concourse.tile import TileContext
from concourse.bass_utils import run_bass_kernel_spmd

N = 100000
D = 256
DEG = 32
K = 8
NCORES = 8
SHARD = N // NCORES          # 12500
RP = 98                      # columns per partition (row wrap): 128*98 = 12544
PADROWS = 128 * RP           # 12544
NCHUNK = 25                  # K1 node chunks of 512
CW = 512

_cache = {}


def _build_k1():
    """s = X @ [w_self|w_neigh] with X, W split into fp16 pairs:
    x = xh+xm (11+11 mantissa bits, residual ~2^-22|x|). PE fp16 products are
    exact in fp32; three cross terms (hh, hm, mh) give ~fp32-dot accuracy."""
    nc = bacc.Bacc(None, target_bir_lowering=False)
    xs = [
        nc.dram_tensor(f"x{s}", [D, SHARD], mybir.dt.float16, kind="ExternalInput")
        for s in range(2)
    ]
    # w8[:, (k*2+s)*2:(k*2+s)*2+2] = split s of W chunk k  (k in 0..1)
    w8 = nc.dram_tensor("w8", [128, 8], mybir.dt.float16, kind="ExternalInput")
    spack = nc.dram_tensor("spack", [2, SHARD], mybir.dt.float32, kind="ExternalOutput")
    PAIRS = [(0, 0), (0, 1), (1, 0)]

    with TileContext(nc) as tc:
        with (
            tc.tile_pool(name="wp", bufs=1) as wp,
            tc.tile_pool(name="xp", bufs=8) as xp,
            tc.tile_pool(name="pp", bufs=6, space="PSUM") as pp,
            tc.tile_pool(name="sp", bufs=1) as sp,
        ):
            w_sb = wp.tile([128, 8], mybir.dt.float16)
            nc.sync.dma_start(w_sb[:], w8[:])
            s_sb = sp.tile([2, NCHUNK * CW], mybir.dt.float32)
            for j in range(NCHUNK):
                w = min(CW, SHARD - j * CW)
                xt_sb = []
                for s in range(2):
                    t0 = xp.tile([128, CW], mybir.dt.float16, tag=f"x{s}a")
                    t1 = xp.tile([128, CW], mybir.dt.float16, tag=f"x{s}b")
                    nc.sync.dma_start(t0[:, :w], xs[s][0:128, j * CW : j * CW + w])
                    nc.sync.dma_start(t1[:, :w], xs[s][128:256, j * CW : j * CW + w])
                    xt_sb.append((t0, t1))
                psum = pp.tile([2, CW], mybir.dt.float32)
                nmm = 2 * len(PAIRS)
                i = 0
                for sx, sw in PAIRS:
                    for k in range(2):
                        c = (k * 2 + sw) * 2
                        nc.tensor.matmul(
                            psum[:, :w],
                            lhsT=w_sb[:, c : c + 2],
                            rhs=xt_sb[sx][k][:, :w],
                            start=(i == 0),
                            stop=(i == nmm - 1),
                        )
                        i += 1
                nc.vector.tensor_copy(s_sb[:, j * CW : j * CW + w], psum[:, :w])
            nc.sync.dma_start(spack[:], s_sb[:, :SHARD])
    nc.compile()
    return nc


def _build_k2():
    F = RP * DEG  # 3136 gathered values per partition
    nc = bacc.Bacc(None, target_bir_lowering=False)
    gat = nc.dram_tensor("gat", [128, F], mybir.dt.float32, kind="ExternalInput")
    sself = nc.dram_tensor("sself", [128, RP], mybir.dt.float32, kind="ExternalInput")
    fcb = nc.dram_tensor("fcb", [128, 1], mybir.dt.float32, kind="ExternalInput")
    nbr = nc.dram_tensor("nbr", [128, F], mybir.dt.int32, kind="ExternalInput")
    iot = nc.dram_tensor("iot", [128, DEG], mybir.dt.float32, kind="ExternalInput")
    ids = nc.dram_tensor("ids", [128, RP * K], mybir.dt.int32, kind="ExternalOutput")
    sc = nc.dram_tensor("sc", [128, RP * K], mybir.dt.float32, kind="ExternalOutput")

    with TileContext(nc) as tc:
        with tc.tile_pool(name="sb", bufs=1) as sb:
            gat_sb = sb.tile([128, F], mybir.dt.float32)
            sself_sb = sb.tile([128, RP], mybir.dt.float32)
            fcb_sb = sb.tile([128, 1], mybir.dt.float32)
            nbrf = sb.tile([128, F], mybir.dt.float32)
            iota_sb = sb.tile([128, DEG], mybir.dt.float32)
            bias = sb.tile([128, RP], mybir.dt.float32)
            lr = sb.tile([128, F], mybir.dt.float32)
            maxs = sb.tile([128, RP * K], mybir.dt.float32)
            posu = sb.tile([128, RP * K], mybir.dt.uint32)
            posf = sb.tile([128, RP * K], mybir.dt.float32)
            oh = sb.tile([128, RP * K * DEG], mybir.dt.float32)
            idsf = sb.tile([128, RP * K], mybir.dt.float32)
            ids_sb = sb.tile([128, RP * K], mybir.dt.int32)
            sc_sb = sb.tile([128, RP * K], mybir.dt.float32)
            nbr_sb = sb.tile([128, F], mybir.dt.int32)

            nc.sync.dma_start(gat_sb[:], gat[:])
            nc.sync.dma_start(sself_sb[:], sself[:])
            nc.sync.dma_start(fcb_sb[:], fcb[:])
            nc.sync.dma_start(nbr_sb[:], nbr[:])
            nc.sync.dma_start(iota_sb[:], iot[:])

            nc.vector.tensor_copy(nbrf[:], nbr_sb[:])  # i32 -> f32 (exact, <2^24)
            nc.vector.tensor_tensor(
                out=bias[:], in0=sself_sb[:],
                in1=fcb_sb[:].to_broadcast([128, RP]),
                op=mybir.AluOpType.add,
            )
            # raw = gathered + bias ;  lr = max(raw, 0.01*raw)
            nc.vector.tensor_tensor(
                out=lr[:].rearrange("p (t j) -> p t j", j=DEG),
                in0=gat_sb[:].rearrange("p (t j) -> p t j", j=DEG),
                in1=bias[:].rearrange("p (t o) -> p t o", o=1).to_broadcast([128, RP, DEG]),
                op=mybir.AluOpType.add,
            )
            nc.vector.scalar_tensor_tensor(
                out=lr[:], in0=lr[:], scalar=0.01, in1=lr[:],
                op0=mybir.AluOpType.mult, op1=mybir.AluOpType.max,
            )
            for t in range(RP):
                nc.vector.max(maxs[:, t * K : (t + 1) * K], lr[:, t * DEG : (t + 1) * DEG])
                nc.vector.max_index(
                    posu[:, t * K : (t + 1) * K],
                    maxs[:, t * K : (t + 1) * K],
                    lr[:, t * DEG : (t + 1) * DEG],
                )
            nc.scalar.activation(sc_sb[:], maxs[:], mybir.ActivationFunctionType.Exp)
            nc.vector.tensor_copy(posf[:], posu[:])  # u32 -> f32
            oh4 = oh[:].rearrange("p (t k j) -> p t k j", k=K, j=DEG)
            posb = posf[:].rearrange("p (t k o) -> p t k o", k=K, o=1).to_broadcast(
                [128, RP, K, DEG]
            )
            iotb = iota_sb[:].rearrange("p (t k j) -> p t k j", t=1, k=1).to_broadcast(
                [128, RP, K, DEG]
            )
            nbrb = nbrf[:].rearrange("p (t k j) -> p t k j", k=1, j=DEG).to_broadcast(
                [128, RP, K, DEG]
            )
            nc.vector.tensor_tensor(
                out=oh4, in0=posb, in1=iotb, op=mybir.AluOpType.is_equal,
            )
            nc.vector.tensor_tensor(
                out=oh4, in0=oh4, in1=nbrb, op=mybir.AluOpType.mult,
            )
            nc.vector.tensor_reduce(
                out=idsf[:].rearrange("p (t k) -> p t k", k=K),
                in_=oh4, axis=mybir.AxisListType.X, op=mybir.AluOpType.add,
            )
            nc.vector.tensor_copy(ids_sb[:], idsf[:])  # f32 -> i32 (exact ints)
            nc.sync.dma_start(ids[:], ids_sb[:])
            nc.sync.dma_start(sc[:], sc_sb[:])
    nc.compile()
    return nc


def _get_kernels():
    if "k1" not in _cache:
        _cache["k1"] = _build_k1()
        _cache["k2"] = _build_k2()
    return _cache["k1"], _cache["k2"]


def _fp16_pair(a):
    hi = a.astype(np.float16)
    mid = (a - hi.astype(np.float32)).astype(np.float16)
    return hi, mid


def _k1_inputs(node_features, fc_w):
    W = np.stack([fc_w[:D], fc_w[D:]], axis=1)  # [256, 2]
    wh, wm = _fp16_pair(W)
    w8_cols = []
    for k in range(2):
        for ws in (wh, wm):
            w8_cols.append(ws[k * 128 : (k + 1) * 128])
    w8 = np.ascontiguousarray(np.concatenate(w8_cols, axis=1))  # [128, 8] fp16
    in1 = []
    for c in range(NCORES):
        xt = np.ascontiguousarray(node_features[c * SHARD : (c + 1) * SHARD].T)
        xh, xm = _fp16_pair(xt)
        in1.append({"x0": xh, "x1": xm, "w8": w8})
    return in1


def kernel(node_features, fc_w, fc_b, neighbors):
    node_features = np.ascontiguousarray(np.asarray(node_features, dtype=np.float32))
    fc_w = np.asarray(fc_w, dtype=np.float32)
    fc_b = np.asarray(fc_b, dtype=np.float32)
    nbr_in_dtype = np.asarray(neighbors).dtype
    neighbors = np.asarray(neighbors, dtype=np.int32)

    k1, k2 = _get_kernels()

    # --- K1: per-core matvec, features on partitions, exact bf16 triples ---
    in1 = _k1_inputs(node_features, fc_w)
    res1 = run_bass_kernel_spmd(k1, in1, core_ids=list(range(NCORES)))
    spacks = [res1.results[c]["spack"] for c in range(NCORES)]
    s_full = np.concatenate(spacks, axis=1)             # [2, 100000]
    s_self_all, s_neigh = s_full[0], s_full[1]

    # host: expand device-computed s_neigh over the neighbor index map
    gathered = s_neigh[neighbors]                       # [N, 32] f32

    # --- K2: scoring + exact top-8 + id extraction ---
    iot = np.tile(np.arange(DEG, dtype=np.float32), (128, 1))
    fcb = np.full((128, 1), fc_b[0], dtype=np.float32)
    in2 = []
    for c in range(NCORES):
        sl = slice(c * SHARD, (c + 1) * SHARD)
        g = np.zeros((PADROWS, DEG), dtype=np.float32)
        g[:SHARD] = gathered[sl]
        nb = np.zeros((PADROWS, DEG), dtype=np.int32)
        nb[:SHARD] = neighbors[sl]
        ss = np.zeros(PADROWS, dtype=np.float32)
        ss[:SHARD] = s_self_all[sl]
        in2.append(
            {
                "gat": g.reshape(128, RP * DEG),
                "nbr": nb.reshape(128, RP * DEG),
                "sself": ss.reshape(128, RP),
                "fcb": fcb,
                "iot": iot,
            }
        )
    res2 = run_bass_kernel_spmd(k2, in2, core_ids=list(range(NCORES)))

    top_ids = np.empty((N, K), dtype=np.int32)
    top_scores = np.empty((N, K), dtype=np.float32)
    for c in range(NCORES):
        sl = slice(c * SHARD, (c + 1) * SHARD)
        top_ids[sl] = res2.results[c]["ids"].reshape(PADROWS, K)[:SHARD]
        top_scores[sl] = res2.results[c]["sc"].reshape(PADROWS, K)[:SHARD]

    return top_ids.astype(nbr_in_dtype), top_scores


# revision 11
# speedup vs baseline: 1.0007x; 1.0007x over previous
"""nn_NeighborSelection Trainium2 kernel.

Pipeline (8 NeuronCores, nodes sharded 12500/core):
  K1 (device): s_self/s_neigh matvec  — node_features @ [w_self|w_neigh]
               via PE fp32 matmuls, features on partitions.
  host:        concat per-core s shards; expand s_neigh[neighbors]
               (pure index permutation of a device-computed vector; no FLOPs).
  K2 (device): raw = gathered + s_self + b; leaky_relu via max(x, 0.01x);
               exact top-8 per row via DVE max8/max_index (jax tie order);
               top_scores = exp(selected) on ACT; ids extracted with a
               one-hot reduce over the 32 neighbor slots.

Selection runs on pre-exp values (exact fp32) — exp is monotone so the
top-k set/order matches the reference, and scores are exp'd after.
"""

import numpy as np

import concourse.bass as bass
import concourse.bacc as bacc
import concourse.mybir as mybir
from concourse.tile import TileContext
from concourse.bass_utils import run_bass_kernel_spmd

N = 100000
D = 256
DEG = 32
K = 8
NCORES = 8
SHARD = N // NCORES          # 12500
RP = 98                      # columns per partition (row wrap): 128*98 = 12544
PADROWS = 128 * RP           # 12544
NCHUNK = 25                  # K1 node chunks of 512
CW = 512

_cache = {}


def _build_k1():
    """s = X @ [w_self|w_neigh] with X, W split into fp16 pairs:
    x = xh+xm (11+11 mantissa bits, residual ~2^-22|x|). PE fp16 products are
    exact in fp32; three cross terms (hh, hm, mh) give ~fp32-dot accuracy."""
    nc = bacc.Bacc(None, target_bir_lowering=False)
    xs = [
        nc.dram_tensor(f"x{s}", [D, SHARD], mybir.dt.float16, kind="ExternalInput")
        for s in range(2)
    ]
    # w8[:, (k*2+s)*2:(k*2+s)*2+2] = split s of W chunk k  (k in 0..1)
    w8 = nc.dram_tensor("w8", [128, 8], mybir.dt.float16, kind="ExternalInput")
    spack = nc.dram_tensor("spack", [2, SHARD], mybir.dt.float32, kind="ExternalOutput")
    PAIRS = [(0, 0), (0, 1), (1, 0)]

    with TileContext(nc) as tc:
        with (
            tc.tile_pool(name="wp", bufs=1) as wp,
            tc.tile_pool(name="xp", bufs=8) as xp,
            tc.tile_pool(name="pp", bufs=6, space="PSUM") as pp,
            tc.tile_pool(name="sp", bufs=1) as sp,
        ):
            w_sb = wp.tile([128, 8], mybir.dt.float16)
            nc.sync.dma_start(w_sb[:], w8[:])
            s_sb = sp.tile([2, NCHUNK * CW], mybir.dt.float32)
            for j in range(NCHUNK):
                w = min(CW, SHARD - j * CW)
                xt_sb = []
                for s in range(2):
                    t0 = xp.tile([128, CW], mybir.dt.float16, tag=f"x{s}a")
                    t1 = xp.tile([128, CW], mybir.dt.float16, tag=f"x{s}b")
                    nc.sync.dma_start(t0[:, :w], xs[s][0:128, j * CW : j * CW + w])
                    nc.sync.dma_start(t1[:, :w], xs[s][128:256, j * CW : j * CW + w])
                    xt_sb.append((t0, t1))
                psum = pp.tile([2, CW], mybir.dt.float32)
                nmm = 2 * len(PAIRS)
                i = 0
                for sx, sw in PAIRS:
                    for k in range(2):
                        c = (k * 2 + sw) * 2
                        nc.tensor.matmul(
                            psum[:, :w],
                            lhsT=w_sb[:, c : c + 2],
                            rhs=xt_sb[sx][k][:, :w],
                            start=(i == 0),
                            stop=(i == nmm - 1),
                        )
                        i += 1
                nc.vector.tensor_copy(s_sb[:, j * CW : j * CW + w], psum[:, :w])
            nc.sync.dma_start(spack[:], s_sb[:, :SHARD])
    nc.compile()
    return nc


def _build_k2():
    F = RP * DEG  # 3136 gathered values per partition
    nc = bacc.Bacc(None, target_bir_lowering=False)
    gat = nc.dram_tensor("gat", [128, F], mybir.dt.float32, kind="ExternalInput")
    sself = nc.dram_tensor("sself", [128, RP], mybir.dt.float32, kind="ExternalInput")
    fcb = nc.dram_tensor("fcb", [128, 1], mybir.dt.float32, kind="ExternalInput")
    nbr = nc.dram_tensor("nbr", [128, F], mybir.dt.int32, kind="ExternalInput")
    iot = nc.dram_tensor("iot", [128, DEG], mybir.dt.float32, kind="ExternalInput")
    ids = nc.dram_tensor("ids", [128, RP * K], mybir.dt.int32, kind="ExternalOutput")
    sc = nc.dram_tensor("sc", [128, RP * K], mybir.dt.float32, kind="ExternalOutput")

    with TileContext(nc) as tc:
        with tc.tile_pool(name="sb", bufs=1) as sb:
            gat_sb = sb.tile([128, F], mybir.dt.float32)
            sself_sb = sb.tile([128, RP], mybir.dt.float32)
            fcb_sb = sb.tile([128, 1], mybir.dt.float32)
            nbrf = sb.tile([128, F], mybir.dt.float32)
            iota_sb = sb.tile([128, DEG], mybir.dt.float32)
            bias = sb.tile([128, RP], mybir.dt.float32)
            lr = sb.tile([128, F], mybir.dt.float32)
            maxs = sb.tile([128, RP * K], mybir.dt.float32)
            posu = sb.tile([128, RP * K], mybir.dt.uint32)
            posf = sb.tile([128, RP * K], mybir.dt.float32)
            oh = sb.tile([128, RP * K * DEG], mybir.dt.float32)
            idsf = sb.tile([128, RP * K], mybir.dt.float32)
            ids_sb = sb.tile([128, RP * K], mybir.dt.int32)
            sc_sb = sb.tile([128, RP * K], mybir.dt.float32)
            nbr_sb = sb.tile([128, F], mybir.dt.int32)

            nc.sync.dma_start(sself_sb[:], sself[:])
            nc.sync.dma_start(fcb_sb[:], fcb[:])
            nc.sync.dma_start(iota_sb[:], iot[:])
            nc.vector.tensor_tensor(
                out=bias[:], in0=sself_sb[:],
                in1=fcb_sb[:].to_broadcast([128, RP]),
                op=mybir.AluOpType.add,
            )
            # front end in halves: second half's DMA streams under the first
            # half's DVE work
            HALVES = [(0, 49), (49, RP)]
            for lo, hi in HALVES:
                cl, ch = lo * DEG, hi * DEG
                nc.sync.dma_start(gat_sb[:, cl:ch], gat[:, cl:ch])
                nc.sync.dma_start(nbr_sb[:, cl:ch], nbr[:, cl:ch])
                nc.vector.tensor_copy(nbrf[:, cl:ch], nbr_sb[:, cl:ch])  # i32->f32
                # raw = gathered + bias ;  lr = max(raw, 0.01*raw)
                nc.vector.tensor_tensor(
                    out=lr[:, cl:ch].rearrange("p (t j) -> p t j", j=DEG),
                    in0=gat_sb[:, cl:ch].rearrange("p (t j) -> p t j", j=DEG),
                    in1=bias[:, lo:hi].rearrange("p (t o) -> p t o", o=1).to_broadcast(
                        [128, hi - lo, DEG]
                    ),
                    op=mybir.AluOpType.add,
                )
                nc.vector.scalar_tensor_tensor(
                    out=lr[:, cl:ch], in0=lr[:, cl:ch], scalar=0.01, in1=lr[:, cl:ch],
                    op0=mybir.AluOpType.mult, op1=mybir.AluOpType.max,
                )
                for t in range(lo, hi):
                    nc.vector.max(
                        maxs[:, t * K : (t + 1) * K], lr[:, t * DEG : (t + 1) * DEG]
                    )
                    nc.vector.max_index(
                        posu[:, t * K : (t + 1) * K],
                        maxs[:, t * K : (t + 1) * K],
                        lr[:, t * DEG : (t + 1) * DEG],
                    )
            nc.scalar.activation(sc_sb[:], maxs[:], mybir.ActivationFunctionType.Exp)
            nc.vector.tensor_copy(posf[:], posu[:])  # u32 -> f32
            oh4 = oh[:].rearrange("p (t k j) -> p t k j", k=K, j=DEG)
            posb = posf[:].rearrange("p (t k o) -> p t k o", k=K, o=1).to_broadcast(
                [128, RP, K, DEG]
            )
            iotb = iota_sb[:].rearrange("p (t k j) -> p t k j", t=1, k=1).to_broadcast(
                [128, RP, K, DEG]
            )
            nbrb = nbrf[:].rearrange("p (t k j) -> p t k j", k=1, j=DEG).to_broadcast(
                [128, RP, K, DEG]
            )
            nc.vector.tensor_tensor(
                out=oh4, in0=posb, in1=iotb, op=mybir.AluOpType.is_equal,
            )
            nc.vector.tensor_tensor(
                out=oh4, in0=oh4, in1=nbrb, op=mybir.AluOpType.mult,
            )
            nc.vector.tensor_reduce(
                out=idsf[:].rearrange("p (t k) -> p t k", k=K),
                in_=oh4, axis=mybir.AxisListType.X, op=mybir.AluOpType.add,
            )
            nc.vector.tensor_copy(ids_sb[:], idsf[:])  # f32 -> i32 (exact ints)
            nc.sync.dma_start(ids[:], ids_sb[:])
            nc.sync.dma_start(sc[:], sc_sb[:])
    nc.compile()
    return nc


def _get_kernels():
    if "k1" not in _cache:
        _cache["k1"] = _build_k1()
        _cache["k2"] = _build_k2()
    return _cache["k1"], _cache["k2"]


def _fp16_pair(a):
    hi = a.astype(np.float16)
    mid = (a - hi.astype(np.float32)).astype(np.float16)
    return hi, mid


def _k1_inputs(node_features, fc_w):
    W = np.stack([fc_w[:D], fc_w[D:]], axis=1)  # [256, 2]
    wh, wm = _fp16_pair(W)
    w8_cols = []
    for k in range(2):
        for ws in (wh, wm):
            w8_cols.append(ws[k * 128 : (k + 1) * 128])
    w8 = np.ascontiguousarray(np.concatenate(w8_cols, axis=1))  # [128, 8] fp16
    in1 = []
    for c in range(NCORES):
        xt = np.ascontiguousarray(node_features[c * SHARD : (c + 1) * SHARD].T)
        xh, xm = _fp16_pair(xt)
        in1.append({"x0": xh, "x1": xm, "w8": w8})
    return in1


def kernel(node_features, fc_w, fc_b, neighbors):
    node_features = np.ascontiguousarray(np.asarray(node_features, dtype=np.float32))
    fc_w = np.asarray(fc_w, dtype=np.float32)
    fc_b = np.asarray(fc_b, dtype=np.float32)
    nbr_in_dtype = np.asarray(neighbors).dtype
    neighbors = np.asarray(neighbors, dtype=np.int32)

    k1, k2 = _get_kernels()

    # --- K1: per-core matvec, features on partitions, exact bf16 triples ---
    in1 = _k1_inputs(node_features, fc_w)
    res1 = run_bass_kernel_spmd(k1, in1, core_ids=list(range(NCORES)))
    spacks = [res1.results[c]["spack"] for c in range(NCORES)]
    s_full = np.concatenate(spacks, axis=1)             # [2, 100000]
    s_self_all, s_neigh = s_full[0], s_full[1]

    # host: expand device-computed s_neigh over the neighbor index map
    gathered = s_neigh[neighbors]                       # [N, 32] f32

    # --- K2: scoring + exact top-8 + id extraction ---
    iot = np.tile(np.arange(DEG, dtype=np.float32), (128, 1))
    fcb = np.full((128, 1), fc_b[0], dtype=np.float32)
    in2 = []
    for c in range(NCORES):
        sl = slice(c * SHARD, (c + 1) * SHARD)
        g = np.zeros((PADROWS, DEG), dtype=np.float32)
        g[:SHARD] = gathered[sl]
        nb = np.zeros((PADROWS, DEG), dtype=np.int32)
        nb[:SHARD] = neighbors[sl]
        ss = np.zeros(PADROWS, dtype=np.float32)
        ss[:SHARD] = s_self_all[sl]
        in2.append(
            {
                "gat": g.reshape(128, RP * DEG),
                "nbr": nb.reshape(128, RP * DEG),
                "sself": ss.reshape(128, RP),
                "fcb": fcb,
                "iot": iot,
            }
        )
    res2 = run_bass_kernel_spmd(k2, in2, core_ids=list(range(NCORES)))

    top_ids = np.empty((N, K), dtype=np.int32)
    top_scores = np.empty((N, K), dtype=np.float32)
    for c in range(NCORES):
        sl = slice(c * SHARD, (c + 1) * SHARD)
        top_ids[sl] = res2.results[c]["ids"].reshape(PADROWS, K)[:SHARD]
        top_scores[sl] = res2.results[c]["sc"].reshape(PADROWS, K)[:SHARD]

    return top_ids.astype(nbr_in_dtype), top_scores
